# revision 2
# baseline (speedup 1.0000x reference)
"""Gaussian kernel matrix (pairwise L2 over T) for x:(32,64,1000,16) -> (32,64,64,16).

out[n,c,d,f] = exp(-||x[n,c,:,f] - x[n,d,:,f]||^2 / 2)

Strategy (8 NeuronCores, data-parallel over N, 4 batch elems per core):
  Streaming pipeline per core, 2 pairs of batch elems (2n x 64c = 128 partitions):
    1. Input DMAd in 8 chunks of [128, 256t, 16f] (fp32->bf16 SWDGE cast, 2MB
       reads, contiguous 16KB per partition) so compute starts after ~2MB.
    2. Per 128-t sub-chunk: 16 PE transposes [128,t]->[t,128] (per f) staged
       through PSUM bf16, copied to SBUF split at the PSUM bank boundary
       (DVE f0:8 / ACT f8:16) so copies overlap transposes bank-disjointly.
    3. Gram matmuls accumulate in PSUM fp32 across the 8 t-chunks per pair
       (K=104 on the tail chunk -- no padding/memset). One [K,M=128,N=128]
       matmul per (f, chunk); diagonal 64x64 blocks are the per-n grams.
    4. Epilogue: diag blocks copied to SBUF (frees PSUM fast); sq_c via
       stacked-identity mask + row reduce; H = exp((G - sq_c)/2);
       H^T within each 64-block via DVE StreamTranspose (32x32 blocks with
       block-index swap); O = H * H^T. Diagonal is exactly 1.
    5. fp32 out per pair via HWDGE (sync) DMA, overlapped with the other pair.
bf16 matmul inputs with fp32 PSUM accumulation; the epilogue's exact diagonal
cancellation makes the output independent of the bf16 rounding on-diagonal.
"""

import numpy as np

N_FULL, C, T, F = 32, 64, 1000, 16
N_CORES = 8
N_PER_CORE = N_FULL // N_CORES  # 4
NPAIRS = N_PER_CORE // 2        # 2
DCH_W = 256                     # dma chunk width in t
NDCH = 4                        # dma chunks per pair (256,256,256,232)
TCH = 8                         # transpose chunks of 128 t per pair (tail=104)

_CACHE = {}


def _split_multi_waits(bir_bytes):
    """Walrus codegen here only supports one sync-wait per instruction; Tile
    emits several. Split extras into preceding NoOp instructions on the same
    engine queue (engine executes in order, so the waits still gate)."""
    import json

    bir = json.loads(bir_bytes)
    cnt = 0
    for fn in bir["functions"]:
        for blk in fn["blocks"]:
            new = []
            for inst in blk["instructions"]:
                si = inst.get("sync_info")
                waits = (si or {}).get("on_wait", [])
                if len(waits) > 1:
                    for w in waits[:-1]:
                        cnt += 1
                        new.append(
                            {
                                "debug": inst.get("debug", 0),
                                "engine": inst["engine"],
                                "ins": [],
                                "outs": [],
                                "name": f"WS{cnt}",
                                "opcode": "NoOp",
                                "sync_info": {"on_update": [], "on_wait": [w]},
                            }
                        )
                    si["on_wait"] = waits[-1:]
                new.append(inst)
            blk["instructions"] = new
    return json.dumps(bir).encode()


def _build_nc():
    import concourse.bass as bass
    import concourse.mybir as mybir
    import concourse.tile as tile
    from concourse.masks import make_identity

    dt = mybir.dt
    nc = bass.Bass()
    x = nc.dram_tensor("x", (N_PER_CORE, C, T, F), dt.float32, kind="ExternalInput")
    y = nc.dram_tensor("y", (N_PER_CORE, C, C, F), dt.float32, kind="ExternalOutput")

    with tile.TileContext(nc) as tc:
        with (
            tc.tile_pool(name="const", bufs=1) as constp,
            tc.tile_pool(name="chunk", bufs=6) as chunkp,
            tc.tile_pool(name="trT", bufs=3) as trp,
            tc.tile_pool(name="work", bufs=2) as workp,
            tc.tile_pool(name="osb", bufs=2) as outp,
            tc.tile_pool(name="ps_tr", bufs=2, space="PSUM") as ps_tr,
            tc.tile_pool(name="ps_gram", bufs=1, space="PSUM") as ps_gram,
        ):
            # --- input DMA chunk issue (pair 0 first, constants, pair 1) ---
            chunks = [[None] * NDCH for _ in range(NPAIRS)]

            def issue_loads(p):
                src = x[2 * p : 2 * p + 2].rearrange("n c t f -> (n c) t f")
                for d in range(NDCH):
                    t0 = d * DCH_W
                    w = min(DCH_W, T - t0)
                    ck = chunkp.tile([128, DCH_W, F], dt.bfloat16, tag="chunk")
                    nc.gpsimd.dma_start(ck[:, :w, :], src[:, t0 : t0 + w, :])
                    chunks[p][d] = ck

            issue_loads(0)

            ident_bf = constp.tile([128, 128], dt.bfloat16)
            make_identity(nc, ident_bf)
            # mask[p, d] = 1.0 iff p % 64 == d (two stacked 64-identities)
            mask = constp.tile([128, 64], dt.float32)
            nc.gpsimd.memset(mask, 0.0)
            for half in range(2):
                nc.gpsimd.affine_select(
                    out=mask,
                    in_=mask,
                    compare_op=mybir.AluOpType.not_equal,
                    fill=1.0,
                    base=-64 * half,
                    pattern=[[-1, 64]],
                    channel_multiplier=1,
                )

            issue_loads(1)

            for p in range(NPAIRS):
                gram = ps_gram.tile([128, F, 128], dt.float32, tag="gram")
                for ch in range(TCH):
                    dch, off = ch // 2, (ch % 2) * 128
                    w = min(128, T - ch * 128)  # 104 on the tail chunk
                    ck = chunks[p][dch]
                    ps = ps_tr.tile([128, F, 128], dt.bfloat16, tag="pstr")
                    for f in range(F):
                        nc.tensor.transpose(
                            ps[:w, f, :], ck[:, off : off + w, f], ident_bf
                        )
                    # split at the PSUM bank boundary (f=8) so the DVE copy of
                    # bank 0 overlaps PE transposes still writing bank 1
                    trT = trp.tile([128, F, 128], dt.bfloat16, tag="trT")
                    nc.vector.tensor_copy(trT[:w, 0:8, :], ps[:w, 0:8, :])
                    nc.scalar.copy(trT[:w, 8:16, :], ps[:w, 8:16, :])
                    for f in range(F):
                        nc.tensor.matmul(
                            gram[:, f, :],
                            trT[:w, f, :],
                            trT[:w, f, :],
                            start=(ch == 0),
                            stop=(ch == TCH - 1),
                            skip_group_check=True,
                        )

                # --- epilogue: drain diag blocks out of PSUM fast ---
                sbG = workp.tile([128, F, 64], dt.float32, tag="sbG")
                nc.vector.tensor_copy(sbG[0:64], gram[0:64, :, 0:64])
                nc.scalar.copy(sbG[64:128], gram[64:128, :, 64:128])

                masked = workp.tile([128, F, 64], dt.float32, tag="masked")
                sq = workp.tile([128, F], dt.float32, tag="sq")
                dti = workp.tile([128, F, 64], dt.float32, tag="dti")
                h = workp.tile([128, F, 64], dt.bfloat16, tag="h")
                hT = workp.tile([128, F, 64], dt.bfloat16, tag="hT")
                nc.vector.tensor_tensor(
                    masked,
                    sbG,
                    mask[:, None, :].to_broadcast((128, F, 64)),
                    mybir.AluOpType.mult,
                )
                nc.vector.reduce_sum(sq, masked, axis=mybir.AxisListType.X)
                nc.vector.tensor_tensor(
                    dti,
                    sbG,
                    sq[:, :, None].to_broadcast((128, F, 64)),
                    mybir.AluOpType.subtract,
                )
                nc.scalar.activation(
                    h, dti, mybir.ActivationFunctionType.Exp, scale=0.5
                )
                # H^T within each 64-block: 32x32 DVE stream transposes with
                # swapped block indices
                for base in (0, 64):
                    for i in range(2):
                        for j in range(2):
                            nc.vector.transpose(
                                hT[base + 32 * i : base + 32 * i + 32, :, 32 * j : 32 * j + 32],
                                h[base + 32 * j : base + 32 * j + 32, :, 32 * i : 32 * i + 32],
                            )
                out_sb = outp.tile([128, C, F], dt.float32, tag="osb")
                nc.vector.tensor_tensor(
                    out_sb.rearrange("p d f -> p f d"),
                    h,
                    hT,
                    mybir.AluOpType.mult,
                )
                dst = y[2 * p : 2 * p + 2].rearrange("n c d f -> (n c) d f")
                nc.sync.dma_start(dst, out_sb)

    orig_ser = nc.to_json_bytes
    nc.to_json_bytes = lambda: _split_multi_waits(orig_ser())
    return nc


def _get_nc():
    if "nc" not in _CACHE:
        _CACHE["nc"] = _build_nc()
    return _CACHE["nc"]


def kernel(x, _trace=False):
    from concourse.bass_utils import run_bass_kernel_spmd

    x = np.ascontiguousarray(np.asarray(x), dtype=np.float32)
    assert x.shape == (N_FULL, C, T, F), x.shape
    nc = _get_nc()
    in_maps = [
        {"x": np.ascontiguousarray(x[N_PER_CORE * i : N_PER_CORE * (i + 1)])}
        for i in range(N_CORES)
    ]
    res = run_bass_kernel_spmd(nc, in_maps, core_ids=list(range(N_CORES)), trace=_trace)
    out = np.concatenate([r["y"] for r in res.results], axis=0)
    if _trace:
        _CACHE["last_result"] = res
    return out
